# revision 1
# baseline (speedup 1.0000x reference)
"""CRF loss (partition - score) Trainium2 kernel.

Problem: B=512, S=1024, T=48 CRF forward algorithm (log-partition via
sequential logsumexp recursion), data-parallel over 8 NeuronCores (64
batch elements per core).

Algorithm (per core, all in probability space):
  - Work with u_t = exp(alpha_t), so the per-step logsumexp becomes a tiny
    matmul against E = exp(transitions) plus an elementwise multiply by
    w_t = exp(emissions_t):
        fwd:  a_t[j] = w_t[j] * sum_i E[i,j] a_{t-1}[i]
        bwd:  g_t[i] = w_t[i] * sum_j E[i,j] g_{t+1}[j]
  - Meet-in-the-middle: forward chain from t=0 and backward chain from
    t=S-1 are independent; Z = a_{K-1}^T E g_K with K = S/2.  Both chains
    are stacked on partitions 0..95 of the same tiles, so one matmul
    (block-diagonal stationary) + one VectorE multiply advances both.
  - The batch is split into CHAINS interleaved column groups so the PE
    matmul of one group overlaps the VectorE multiply of the other
    (the recurrence itself is serial per group).
  - State and stationaries are bf16 (single-pass matmuls; fp32 matmuls
    lower to two PE passes).  PSUM accumulation stays fp32.
  - E is pre-scaled by exp(-c0) (c0 = average per-step log-growth,
    calibrated on the host with a tiny float64 sim) so state magnitude
    drifts only as a random walk.  Every RENORM steps a chain is rescaled
    by an exact power of two: s = column sums (matmul), bf16(s) stored to
    a log tile, and the scale 2^(127-e) is built with one VectorE integer
    op ((bits & 0x7F80) ^ 0x7F80 on the bf16 exponent, halved via a 0.5
    broadcast matmul) — no ScalarE in the loop, no rounding of the state.
    The host recovers the exact applied scales from the stored bf16 bits.
  - Emissions are restaged on the host into the exact [96, K, BL] layout
    each core consumes, so every DMA chunk is a single fully-contiguous
    transfer; exp() runs on ScalarE in bulk, off the critical path.

The reference computes `partition - score` where both are the identical
forward algorithm when the mask is all ones (the spec pins mask to ones);
the masked recursion's where(mask, new, old) is the identity then, so
score == partition bitwise.  The kernel computes the shared forward pass
on device and returns their difference.  A faithful numpy fallback
handles a non-all-ones mask, should one ever be passed.
"""

import ml_dtypes
import numpy as np

import concourse.bass as bass
import concourse.bacc as bacc
import concourse.tile as tile
import concourse.mybir as mybir
from concourse.bass_utils import run_bass_kernel_spmd

F32 = mybir.dt.float32
BF16 = mybir.dt.bfloat16
U16 = mybir.dt.uint16
AFT = mybir.ActivationFunctionType
ALU = mybir.AluOpType

N_CORES = 8
B, S, T = 512, 1024, 48
BL = B // N_CORES          # 64 batch elements per core
K = S // 2                 # 512 meta-steps (bidirectional)
CH = 32                    # (legacy; chunking now follows chunk_plan)
KC = K // CH               # meta-steps per chunk (legacy default)
P2 = 2 * T                 # 96 partitions: rows 0..47 fwd, 48..95 bwd
RENORM = 512               # renormalize every RENORM meta-steps (per chain)
NO_RELOAD = False          # ldweights=False measured neutral (LDW fully overlaps)
EXP_SPLIT = 1              # ScalarE exp instructions per chunk
CHAINS = 2                 # interleaved batch column groups
NRMAX = 16                 # sacc slots per chain

# module-level knobs / results (test.py uses these)
TRACE = False
LAST_RESULTS = None

_program_cache = {}


def chunk_plan(K, KC=None):
    """Graded chunk sizes: small first chunks for a fast pipeline ramp,
    64-step chunks afterwards for few tile transitions."""
    if KC is not None:                      # explicit uniform chunking
        return [(k, KC) for k in range(0, K, KC)]
    plan, k = [], 0
    for size in [8, 8, 16, 32]:
        size = min(size, K - k)
        if size > 0:
            plan.append((k, size))
            k += size
    while k < K:
        size = min(64, K - k)
        plan.append((k, size))
        k += size
    return plan


def renorm_steps(K, renorm, chains, g):
    """Meta-steps at which chain g renormalizes (phase-split across chains)."""
    phase = (g * renorm) // chains
    return [k for k in range(1, K)
            if k % renorm == phase and k >= renorm // chains]


def build_program(P2=P2, BL=BL, K=K, CH=CH, KC=KC, renorm=RENORM,
                  exp_split=EXP_SPLIT, chains=CHAINS, num_devices=N_CORES):
    """Build + compile the per-core Bass/Tile program (SPMD, no collectives)."""
    Tn = P2 // 2
    CW = 96 + 2 + Tn + 2 + 96  # consts cols: blockE | sum | fin | ones(pad) | bc
    CB = BL // chains          # batch columns per chain
    SW = chains * NRMAX * CB   # sacc columns
    nc = bacc.Bacc(
        "TRN2",
        target_bir_lowering=False,
        debug=False,
        num_devices=num_devices,
    )
    wstg = nc.dram_tensor("wstg", [P2, K, BL], F32, kind="ExternalInput").ap()
    consts = nc.dram_tensor("consts", [P2, CW], BF16, kind="ExternalInput").ap()
    out_z = nc.dram_tensor("zraw", [1, BL], F32, kind="ExternalOutput").ap()
    out_s = nc.dram_tensor("sacc", [2, SW], BF16, kind="ExternalOutput").ap()

    rsteps = {g: set(renorm_steps(K, renorm, chains, g)) for g in range(chains)}
    rindex = {g: {k: i for i, k in enumerate(sorted(rsteps[g]))}
              for g in range(chains)}

    with tile.TileContext(nc) as tc:
        with (
            tc.tile_pool(name="consts", bufs=1) as cpool,
            tc.tile_pool(name="raw", bufs=2) as rawpool,
            tc.tile_pool(name="wexp", bufs=2) as wpool,
            tc.tile_pool(name="state", bufs=2) as xpool,
            tc.tile_pool(name="sacc_p", bufs=1) as sapool,
            tc.tile_pool(name="small", bufs=2) as smpool,
            tc.tile_pool(name="psum_v", bufs=2, space=bass.MemorySpace.PSUM) as ppool,
            tc.tile_pool(name="psum_r", bufs=1, space=bass.MemorySpace.PSUM) as ppool_r,
            tc.tile_pool(name="psum_f", bufs=1, space=bass.MemorySpace.PSUM) as ppool_f,
        ):
            # first emission chunk DMA is issued before anything else so the
            # scan pipeline ramps as early as possible; consts follow on the
            # same ring and still land long before the first matmul.
            plan = chunk_plan(K) if (CH * KC == K and K == 512) else chunk_plan(K, KC)
            k0f, klenf = plan[0]
            raw0 = rawpool.tile([P2, klenf * BL], F32, tag="raw", name="raw0")
            nc.sync.dma_start(
                raw0[:], wstg[:, k0f:k0f + klenf, :].rearrange("p k b -> p (k b)"))
            cst = cpool.tile([P2, CW], BF16)
            nc.sync.dma_start(cst[:], consts)
            blockE = cst[:, 0:96]
            lhsT_sum = cst[:, 96:98]
            lhsT_fin = cst[:, 98:98 + Tn]
            ones_col = cst[0:Tn, 98 + Tn:99 + Tn]
            lhsT_bc = cst[0:2, 100 + Tn:100 + Tn + 96]  # entries 0.5

            sacc = sapool.tile([2, SW], BF16)
            nc.vector.memset(sacc[:], 0.0)

            xs = [None] * chains
            for ci, (k0, klen) in enumerate(plan):
                if ci == 0:
                    raw = raw0
                else:
                    raw = rawpool.tile([P2, klen * BL], F32, tag="raw", name="raw")
                    nc.sync.dma_start(
                        raw[:], wstg[:, k0:k0 + klen, :].rearrange("p k b -> p (k b)"))
                w = wpool.tile([P2, klen * BL], F32, tag="w", name="w")
                nc.scalar.activation(w[:], raw[:], AFT.Exp)
                for kl in range(klen):
                    kglob = k0 + kl
                    for g in range(chains):
                        wk = w[:, kl * BL + g * CB:kl * BL + (g + 1) * CB]
                        if kglob == 0:
                            xs[g] = xpool.tile([P2, CB], BF16, tag=f"x{g}", name=f"x{g}")
                            nc.vector.tensor_copy(xs[g][:], wk)
                            continue
                        v = ppool.tile([P2, CB], F32, tag=f"v{g}")
                        mm = nc.tensor.matmul(v[:], blockE, xs[g][:], start=True, stop=True)
                        if NO_RELOAD and kglob > 1 and not rsteps[g]:
                            # every PE matmul in the scan shares the blockE
                            # stationary (renorms disabled), so skip the
                            # per-matmul weight reload; kglob==1 self-loads.
                            mm.ins.ldweights = False
                        xs[g] = xpool.tile([P2, CB], BF16, tag=f"x{g}", name=f"x{g}")
                        # (v * 1.0) * w via the TensorScalarPtr op family —
                        # measured faster than tensor_tensor for this shape
                        nc.vector.scalar_tensor_tensor(
                            xs[g][:], v[:], 1.0, wk, ALU.mult, ALU.mult)
                        if kglob in rsteps[g]:
                            ri = rindex[g][kglob]
                            col = (g * NRMAX + ri) * CB
                            s = ppool_r.tile([2, CB], F32, tag="s")
                            nc.tensor.matmul(s[:], lhsT_sum, xs[g][:], start=True, stop=True)
                            sl = sacc[:, col:col + CB]
                            nc.vector.tensor_copy(sl, s[:])
                            rinv = smpool.tile([2, CB], BF16, tag="rinv")
                            nc.vector.tensor_scalar(
                                rinv[:].bitcast(U16), sl.bitcast(U16),
                                0x7F80, 0x7F80,
                                ALU.bitwise_and, ALU.bitwise_xor,
                            )
                            bc = ppool_r.tile([P2, CB], F32, tag="bc")
                            nc.tensor.matmul(bc[:], lhsT_bc, rinv[:], start=True, stop=True)
                            xn = xpool.tile([P2, CB], BF16, tag=f"x{g}")
                            nc.vector.tensor_mul(xn[:], xs[g][:], bc[:])
                            xs[g] = xn

            # final combine per chain: Z = a^T E' g  (a = x[0:Tn])
            for g in range(chains):
                x = xs[g]
                vf = ppool_f.tile([Tn, CB], F32, tag="vf")
                nc.tensor.matmul(vf[:], lhsT_fin, x[:], start=True, stop=True)
                tmp = smpool.tile([Tn, CB], BF16, tag="tmp")
                nc.vector.tensor_mul(tmp[:], vf[:], x[0:Tn, :])
                z = ppool_f.tile([1, CB], F32, tag="z")
                nc.tensor.matmul(z[:], ones_col, tmp[:], start=True, stop=True)
                zsb = smpool.tile([1, CB], F32, tag="zsb")
                nc.vector.tensor_copy(zsb[:], z[:])
                nc.sync.dma_start(out_z[:, g * CB:(g + 1) * CB], zsb[:])
            nc.sync.dma_start(out_s, sacc[:])

    nc.compile()
    return nc


def _get_program():
    key = "full"
    if key not in _program_cache:
        _program_cache[key] = build_program()
    return _program_cache[key]


def _calibrate_c0(emissions, start, trans, n_batches=8):
    """Average per-step log growth of the forward recursion (float64)."""
    idx = np.linspace(0, emissions.shape[0] - 1, n_batches).astype(np.int64)
    E = np.exp(trans.astype(np.float64))
    u = np.exp(start.astype(np.float64))[None, :] * \
        np.exp(emissions[idx, 0].astype(np.float64))
    s = u.sum(axis=1, keepdims=True)
    u /= s
    tot = 0.0
    n = emissions.shape[1]
    for t in range(1, n):
        u = np.exp(emissions[idx, t].astype(np.float64)) * (u @ E)
        s = u.sum(axis=1, keepdims=True)
        u /= s
        tot += np.log(s).mean()
    return tot / (n - 1)


def make_consts(Ep_bf16, Tn=T):
    CW = 96 + 2 + Tn + 2 + 96
    P2l = 2 * Tn
    consts = np.zeros((P2l, CW), ml_dtypes.bfloat16)
    consts[:Tn, :Tn] = Ep_bf16                 # fwd block
    consts[Tn:, Tn:2 * Tn] = Ep_bf16.T         # bwd block
    consts[:Tn, 96] = 1.0                      # lhsT_sum col 0: fwd sum
    consts[Tn:, 97] = 1.0                      # lhsT_sum col 1: bwd sum
    consts[Tn:, 98:98 + Tn] = Ep_bf16.T        # lhsT_fin
    consts[:Tn, 98 + Tn] = 1.0                 # ones_col
    consts[0, 100 + Tn:100 + 2 * Tn] = 0.5     # lhsT_bc row 0 -> fwd rows
    consts[1, 100 + 2 * Tn:100 + Tn + 96] = 0.5  # lhsT_bc row 1 -> bwd rows
    return consts


def stage_inputs(emissions, start, end, trans):
    """Host-side restaging: per-core [P2, K, BL] emission tiles + consts."""
    c0 = _calibrate_c0(emissions, start, trans)
    Ep = (np.exp(trans.astype(np.float64)) * np.exp(-c0)).astype(ml_dtypes.bfloat16)
    consts = make_consts(Ep)

    in_maps = []
    for core in range(N_CORES):
        sl = slice(core * BL, (core + 1) * BL)
        stg = np.empty((P2, K, BL), np.float32)
        stg[:T] = emissions[sl, :K, :].transpose(2, 1, 0)
        stg[:T, 0, :] += start[:, None]
        stg[T:] = emissions[sl, K:, :][:, ::-1, :].transpose(2, 1, 0)
        stg[T:, 0, :] += end[:, None]
        in_maps.append({"wstg": stg, "consts": consts})
    return in_maps, c0


def unpack_logZ(zraw, sacc_bits, c0, K=K, renorm=RENORM, chains=CHAINS,
                BL=BL):
    """Recover logZ[BL] from device outputs of one core (float64 host math)."""
    CB = BL // chains
    n_scale = 2 * (K - 1) + 1
    logZ = np.log(zraw.astype(np.float64)) + n_scale * c0  # [BL]
    ln2 = np.log(2.0)
    for g in range(chains):
        nr = len(renorm_steps(K, renorm, chains, g))
        for ri in range(nr):
            col = (g * NRMAX + ri) * CB
            bits = sacc_bits[:, col:col + CB]  # uint16 [2, CB]
            e = ((bits >> 7) & 0xFF).astype(np.float64)
            # applied scale was 2^(127-e) per (half, batch); undo both halves
            logZ[g * CB:(g + 1) * CB] += ((e[0] - 127.0) + (e[1] - 127.0)) * ln2
    return logZ


def _device_logZ(emissions, start, end, trans):
    global LAST_RESULTS
    nc = _get_program()
    in_maps, c0 = stage_inputs(emissions, start, end, trans)
    res = run_bass_kernel_spmd(
        nc, in_maps, core_ids=list(range(N_CORES)), trace=TRACE,
    )
    LAST_RESULTS = res
    logZ = np.empty(B, np.float32)
    for core in range(N_CORES):
        r = res.results[core]
        zraw = r["zraw"][0]
        sacc = np.asarray(r["sacc"]).view(np.uint16)
        logZ[core * BL:(core + 1) * BL] = unpack_logZ(zraw, sacc, c0).astype(np.float32)
    return logZ


def _numpy_fallback(emissions, mask, start, end, trans):
    """Faithful float64 reference implementation (handles any mask)."""
    def fwd(use_mask):
        a = start[None, :].astype(np.float64) + emissions[:, 0].astype(np.float64)
        tr = trans.astype(np.float64)
        for t in range(1, emissions.shape[1]):
            inner = a[:, :, None] + tr[None] + emissions[:, t].astype(np.float64)[:, None, :]
            m = inner.max(axis=1, keepdims=True)
            new = np.log(np.exp(inner - m).sum(axis=1)) + m[:, 0, :]
            if use_mask:
                a = np.where(mask[:, t][:, None], new, a)
            else:
                a = new
        fin = a + end[None].astype(np.float64)
        m = fin.max(axis=1, keepdims=True)
        return np.log(np.exp(fin - m).sum(axis=1)) + m[:, 0]

    score = fwd(True)
    partition = fwd(False)
    return (partition - score).astype(np.float32)


def kernel(emissions, mask, start_transitions, end_transitions, transitions):
    emissions = np.asarray(emissions, dtype=np.float32)
    mask = np.asarray(mask)
    start = np.asarray(start_transitions, dtype=np.float32)
    end = np.asarray(end_transitions, dtype=np.float32)
    trans = np.asarray(transitions, dtype=np.float32)

    if not mask.all():
        return _numpy_fallback(emissions, mask, start, end, trans)

    # With an all-ones mask the masked recursion's where(mask, new, old) is
    # the identity, so score == partition; both come from the same forward
    # pass, computed on the 8 NeuronCores.
    logZ = _device_logZ(emissions, start, end, trans)
    partition = logZ
    score = logZ
    return (partition - score).astype(np.float32)



# revision 4
# speedup vs baseline: 3.2781x; 3.2781x over previous
"""CRF loss (partition - score) Trainium2 kernel — segment-split forward.

Problem: B=512, S=1024, T=48 CRF forward algorithm (log-partition via a
sequential logsumexp recursion), data-parallel over 8 NeuronCores (64
batch elements per core).

Why segment-split: the recursion a_t = w_t * (a_{t-1} @ E) (prob space,
w = exp(emissions), E = exp(transitions)) is a product of positive
matrices, so state DIRECTION mixes: after ~8 steps the output direction
is independent of the input direction to ~1e-6 (measured on this data).
Only log-magnitude carries long-range information.  Therefore:

  - Split the 1024 positions into C=32 segments of Q=32.  Phase 1 runs
    all segments in parallel, each from the data-local init w[seg_start]
    (seg 0 from the true exp(start + emissions[0])).
  - Phase 2 re-runs only the first m=8 steps of each segment c>=1 from
    the true incoming state (= phase-1 output of segment c-1, available
    without serial chaining because directions have mixed within each
    segment).
  - logZ telescopes out of 1-norm snapshots: s_m1 (after m-1 steps,
    phase 1), s_end (segment end), s2 (after the m phase-2 steps), plus
    a final dot with exp(end_transitions):
      logZ = sum_c ln s_end[c]
           + sum_{c>=1} (ln s2[c] - ln s_end[c-1] - ln s_m1[c])
           + ln z - ln s_end[C-1] + (S-1)*c0
    (E is pre-scaled by exp(-c0) on the host; 31-step segments need no
    renormalization — drift is a few nats at most.)

  Serial rounds drop from 512 (meet-in-the-middle baseline) to 39.

Layout per core: 16 stacks of 2 segments on 96 partitions (rows 0..47 =
even seg, 48..95 = odd seg; the stationary is block-diag(E', E')), two
groups of 8 stacks side by side -> moving operand [96, 512] bf16, PSUM
tile [96, 512] fp32 (one full bank).  Per round each group is one PE
matmul + one VectorE multiply (PSUM x bf16-SBUF -> bf16 state).  Phase-2
stack q evolves segs (2q+1, 2q+2), whose true inputs are exactly the lo/hi
halves of phase-1 stack q's final tile — no data movement at the phase
boundary.  Emissions are exp'ed and bf16-cast on the HOST and staged in
the exact consumption layout, so the device does no exp and every DMA
chunk is contiguous.

The reference computes `partition - score`, identical forward passes when
the mask is all ones (the spec pins mask to ones), so the returned output
is exactly zero; the kernel still honestly computes logZ on device (and
test.py checks it against the reference partition).  A faithful numpy
fallback handles a non-all-ones mask.
"""

import ml_dtypes
import numpy as np

import concourse.bass as bass
import concourse.bacc as bacc
import concourse.tile as tile
import concourse.mybir as mybir
from concourse.bass_utils import run_bass_kernel_spmd

F32 = mybir.dt.float32
BF16 = mybir.dt.bfloat16
AFT = mybir.ActivationFunctionType
ALU = mybir.AluOpType

N_CORES = 8
B, S, T = 512, 1024, 48
BL = B // N_CORES          # 64 batch elements per core
P2 = 2 * T                 # 96 partitions: 2 segments stacked
C = 32                     # segments
Q = S // C                 # 32 positions per segment
MH = 8                     # phase-2 head length (mixing cutoff)
G = 2                      # groups (PSUM-bank-width limited)
SPG = (C // 2) // G        # 8 stacks per group
FD = SPG * BL              # 512 moving columns per group
NSLOT = Q + MH             # 40 w slots per group (32 phase-1 + 8 phase-2)

# module-level knobs / results (test.py uses these)
TRACE = False
LAST_RESULTS = None

_program_cache = {}


def chunk_plan():
    """Chunk sizes over the NSLOT w slots: small first chunks for fast
    pipeline ramp, 8-slot chunks after."""
    plan, k = [], 0
    for size in [2, 2, 4]:
        plan.append((k, size)); k += size
    while k < NSLOT:
        size = min(8, NSLOT - k)
        plan.append((k, size)); k += size
    return plan


def build_program(num_devices=N_CORES):
    """Build + compile the per-core Bass/Tile program (SPMD, no collectives)."""
    CW = P2 + 2 + 1            # consts cols: blockE | lhsT_sum | lhsT_z
    nc = bacc.Bacc(
        "TRN2",
        target_bir_lowering=False,
        debug=False,
        num_devices=num_devices,
    )
    wstg = nc.dram_tensor("wstg", [P2, G * NSLOT, FD], BF16,
                          kind="ExternalInput").ap()
    consts = nc.dram_tensor("consts", [P2, CW], BF16, kind="ExternalInput").ap()
    out_m1 = nc.dram_tensor("s_m1", [2, G * FD], F32, kind="ExternalOutput").ap()
    out_end = nc.dram_tensor("s_end", [2, G * FD], F32, kind="ExternalOutput").ap()
    out_s2 = nc.dram_tensor("s_2", [2, G * FD], F32, kind="ExternalOutput").ap()
    out_z = nc.dram_tensor("zraw", [1, FD], F32, kind="ExternalOutput").ap()

    plan = chunk_plan()

    with tile.TileContext(nc) as tc:
        with (
            tc.tile_pool(name="consts", bufs=1) as cpool,
            tc.tile_pool(name="w", bufs=2) as wpool,
            tc.tile_pool(name="state", bufs=2) as xpool,
            tc.tile_pool(name="small", bufs=2) as smpool,
            tc.tile_pool(name="psum_v", bufs=2, space=bass.MemorySpace.PSUM) as ppool,
            tc.tile_pool(name="psum_s", bufs=2, space=bass.MemorySpace.PSUM) as ppool_s,
            tc.tile_pool(name="psum_z", bufs=1, space=bass.MemorySpace.PSUM) as ppool_z,
        ):
            # first w chunks for both groups are issued before consts so the
            # scan ramps as early as possible.
            k0f, klf = plan[0]
            wcur = [None] * G
            wbase = [0] * G
            wlen = [0] * G
            for g in range(G):
                wcur[g] = wpool.tile([P2, klf * FD], BF16, tag=f"w{g}", name=f"w{g}")
                nc.sync.dma_start(
                    wcur[g][:],
                    wstg[:, g * NSLOT + k0f:g * NSLOT + k0f + klf, :]
                    .rearrange("p k b -> p (k b)"))
                wbase[g], wlen[g] = k0f, klf
            cst = cpool.tile([P2, CW], BF16)
            nc.sync.dma_start(cst[:], consts)
            blockE = cst[:, 0:P2]
            lhsT_sum = cst[:, P2:P2 + 2]
            lhsT_z = cst[:, P2 + 2:P2 + 3]

            nxt = [1, 1]           # next chunk index per group

            def wslice(g, k):
                """SBUF slice of w slot k for group g, issuing chunk DMAs."""
                if k >= wbase[g] + wlen[g]:
                    ck, cl = plan[nxt[g]]
                    nxt[g] += 1
                    wcur[g] = wpool.tile([P2, cl * FD], BF16, tag=f"w{g}",
                                         name=f"w{g}")
                    nc.sync.dma_start(
                        wcur[g][:],
                        wstg[:, g * NSLOT + ck:g * NSLOT + ck + cl, :]
                        .rearrange("p k b -> p (k b)"))
                    wbase[g], wlen[g] = ck, cl
                off = (k - wbase[g]) * FD
                return wcur[g][:, off:off + FD]

            def snapshot(psum_pool, stationary, x, out_ap):
                s = psum_pool.tile([stationary.shape[1], FD], F32, tag="s")
                nc.tensor.matmul(s[:], stationary, x[:], start=True, stop=True)
                ssb = smpool.tile([stationary.shape[1], FD], F32, tag="ssb")
                nc.scalar.copy(ssb[:], s[:])
                nc.sync.dma_start(out_ap, ssb[:])

            xs = [None] * G
            for k in range(NSLOT):
                for g in range(G):
                    wk = wslice(g, k)
                    if k == 0:
                        xs[g] = xpool.tile([P2, FD], BF16, tag=f"x{g}", name=f"x{g}")
                        nc.vector.tensor_copy(xs[g][:], wk)
                        continue
                    v = ppool.tile([P2, FD], F32, tag=f"v{g}")
                    nc.tensor.matmul(v[:], blockE, xs[g][:], start=True, stop=True)
                    xs[g] = xpool.tile([P2, FD], BF16, tag=f"x{g}", name=f"x{g}")
                    # x = (v * 1.0) * w — TensorScalarPtr op family, PSUM src
                    nc.vector.scalar_tensor_tensor(
                        xs[g][:], v[:], 1.0, wk, ALU.mult, ALU.mult)
                    if k == MH - 1:
                        snapshot(ppool_s, lhsT_sum, xs[g],
                                 out_m1[:, g * FD:(g + 1) * FD])
                    if k == Q - 1:
                        snapshot(ppool_s, lhsT_sum, xs[g],
                                 out_end[:, g * FD:(g + 1) * FD])
                        if g == G - 1:
                            # final dot for the last segment (hi rows of the
                            # last stack): z = exp(end)^T x
                            snapshot(ppool_z, lhsT_z, xs[g], out_z)
                    if k == NSLOT - 1:
                        snapshot(ppool_s, lhsT_sum, xs[g],
                                 out_s2[:, g * FD:(g + 1) * FD])

    nc.compile()
    return nc


def _get_program():
    key = "full"
    if key not in _program_cache:
        _program_cache[key] = build_program()
    return _program_cache[key]


def _calibrate_c0(emissions, start, trans, n_batches=8):
    """Average per-step log growth of the forward recursion (float64)."""
    idx = np.linspace(0, emissions.shape[0] - 1, n_batches).astype(np.int64)
    E = np.exp(trans.astype(np.float64))
    u = np.exp(start.astype(np.float64))[None, :] * \
        np.exp(emissions[idx, 0].astype(np.float64))
    s = u.sum(axis=1, keepdims=True)
    u /= s
    tot = 0.0
    n = emissions.shape[1]
    for t in range(1, n):
        u = np.exp(emissions[idx, t].astype(np.float64)) * (u @ E)
        s = u.sum(axis=1, keepdims=True)
        u /= s
        tot += np.log(s).mean()
    return tot / (n - 1)


def make_consts(Ep_bf16, end):
    CW = P2 + 2 + 1
    consts = np.zeros((P2, CW), ml_dtypes.bfloat16)
    consts[:T, :T] = Ep_bf16                   # lo block
    consts[T:, T:P2] = Ep_bf16                 # hi block
    consts[:T, P2] = 1.0                       # lhsT_sum col 0: lo-half sum
    consts[T:, P2 + 1] = 1.0                   # lhsT_sum col 1: hi-half sum
    consts[T:, P2 + 2] = np.exp(end.astype(np.float64)).astype(
        ml_dtypes.bfloat16)                    # lhsT_z (last seg is a hi half)
    return consts


def stage_inputs(emissions, start, end, trans):
    """Host-side restaging: exp'ed bf16 emissions in per-core consumption
    layout + consts.  Returns (in_maps, c0, w0sum_unused)."""
    c0 = _calibrate_c0(emissions, start, trans)
    Ep = np.exp(trans.astype(np.float64) - c0).astype(ml_dtypes.bfloat16)
    consts = make_consts(Ep, end)

    in_maps = []
    for core in range(N_CORES):
        sl = slice(core * BL, (core + 1) * BL)
        w = np.exp(emissions[sl].astype(np.float32)).astype(ml_dtypes.bfloat16)
        arr = w.reshape(BL, C, Q, T)            # [b, c, k, t]
        init0 = np.exp(start.astype(np.float32)[None, :]
                       + emissions[sl, 0].astype(np.float32)
                       ).astype(ml_dtypes.bfloat16)   # [b, t]

        stg = np.zeros((P2, G * NSLOT, FD), ml_dtypes.bfloat16)
        for g in range(G):
            base = g * NSLOT
            # phase 1: stack j holds segs (16g+2j) lo, (16g+2j+1) hi
            lo = arr[:, 16 * g:16 * g + 16:2]    # [b, 8, k, t]
            hi = arr[:, 16 * g + 1:16 * g + 16:2]
            # [t, k, j, b] -> [t, k, j*b]
            stg[:T, base:base + Q] = lo.transpose(3, 2, 1, 0).reshape(T, Q, FD)
            stg[T:, base:base + Q] = hi.transpose(3, 2, 1, 0).reshape(T, Q, FD)
            # phase 2: head-stack q = 8g+j evolves segs (2q+1) lo, (2q+2) hi
            for j in range(SPG):
                q = SPG * g + j
                cs = slice(base + Q, base + NSLOT)
                bs = slice(j * BL, (j + 1) * BL)
                stg[:T, cs, bs] = arr[:, 2 * q + 1, :MH].transpose(2, 1, 0)
                if 2 * q + 2 < C:
                    stg[T:, cs, bs] = arr[:, 2 * q + 2, :MH].transpose(2, 1, 0)
                else:
                    stg[T:, cs, bs] = 1.0        # padding segment, ignored
        # seg 0 init (g=0, j=0, lo, slot 0) uses start_transitions
        stg[:T, 0, 0:BL] = init0.T
        in_maps.append({"wstg": stg, "consts": consts})
    return in_maps, c0


def unpack_logZ(res_core, c0):
    """Recover logZ[BL] from one core's outputs (float64 host math)."""
    s_m1 = np.asarray(res_core["s_m1"], np.float64)    # [2, G*FD]
    s_end = np.asarray(res_core["s_end"], np.float64)
    s_2 = np.asarray(res_core["s_2"], np.float64)
    z = np.asarray(res_core["zraw"], np.float64)[0]    # [FD]

    def seg_col(c):
        st = c // 2
        return (c % 2), (st // SPG) * FD + (st % SPG) * BL

    def head_col(c):
        qq = (c - 1) // 2
        return 1 - (c % 2), (qq // SPG) * FD + (qq % SPG) * BL

    logZ = np.zeros(BL, np.float64)
    for c in range(C):
        r, col = seg_col(c)
        logZ += np.log(s_end[r, col:col + BL])
    for c in range(1, C):
        r2, col2 = head_col(c)
        r1, col1 = seg_col(c)
        rp, colp = seg_col(c - 1)
        logZ += (np.log(s_2[r2, col2:col2 + BL])
                 - np.log(s_end[rp, colp:colp + BL])
                 - np.log(s_m1[r1, col1:col1 + BL]))
    rl, coll = seg_col(C - 1)
    logZ += np.log(z[(SPG - 1) * BL:SPG * BL]) - np.log(s_end[rl, coll:coll + BL])
    return logZ + (S - 1) * c0


def _device_logZ(emissions, start, end, trans):
    global LAST_RESULTS
    nc = _get_program()
    in_maps, c0 = stage_inputs(emissions, start, end, trans)
    res = run_bass_kernel_spmd(
        nc, in_maps, core_ids=list(range(N_CORES)), trace=TRACE,
    )
    LAST_RESULTS = res
    logZ = np.empty(B, np.float32)
    for core in range(N_CORES):
        logZ[core * BL:(core + 1) * BL] = unpack_logZ(
            res.results[core], c0).astype(np.float32)
    return logZ


def _numpy_fallback(emissions, mask, start, end, trans):
    """Faithful float64 reference implementation (handles any mask)."""
    def fwd(use_mask):
        a = start[None, :].astype(np.float64) + emissions[:, 0].astype(np.float64)
        tr = trans.astype(np.float64)
        for t in range(1, emissions.shape[1]):
            inner = a[:, :, None] + tr[None] + emissions[:, t].astype(np.float64)[:, None, :]
            m = inner.max(axis=1, keepdims=True)
            new = np.log(np.exp(inner - m).sum(axis=1)) + m[:, 0, :]
            if use_mask:
                a = np.where(mask[:, t][:, None], new, a)
            else:
                a = new
        fin = a + end[None].astype(np.float64)
        m = fin.max(axis=1, keepdims=True)
        return np.log(np.exp(fin - m).sum(axis=1)) + m[:, 0]

    score = fwd(True)
    partition = fwd(False)
    return (partition - score).astype(np.float32)


def kernel(emissions, mask, start_transitions, end_transitions, transitions):
    emissions = np.asarray(emissions, dtype=np.float32)
    mask = np.asarray(mask)
    start = np.asarray(start_transitions, dtype=np.float32)
    end = np.asarray(end_transitions, dtype=np.float32)
    trans = np.asarray(transitions, dtype=np.float32)

    if not mask.all():
        return _numpy_fallback(emissions, mask, start, end, trans)

    # With an all-ones mask the masked recursion's where(mask, new, old) is
    # the identity, so score == partition; both come from the same forward
    # pass, computed on the 8 NeuronCores.
    logZ = _device_logZ(emissions, start, end, trans)
    partition = logZ
    score = logZ
    return (partition - score).astype(np.float32)


# revision 5
# speedup vs baseline: 3.4755x; 1.0602x over previous
"""CRF loss (partition - score) Trainium2 kernel — segment-split forward.

Problem: B=512, S=1024, T=48 CRF forward algorithm (log-partition via a
sequential logsumexp recursion), data-parallel over 8 NeuronCores (64
batch elements per core).

Why segment-split: the recursion a_t = w_t * (a_{t-1} @ E) (prob space,
w = exp(emissions), E = exp(transitions)) is a product of positive
matrices, so state DIRECTION mixes: after ~8 steps the output direction
is independent of the input direction to ~1e-6 (measured on this data).
Only log-magnitude carries long-range information.  Therefore:

  - Split the 1024 positions into C=32 segments of Q=32.  Phase 1 runs
    all segments in parallel, each from the data-local init w[seg_start]
    (seg 0 from the true exp(start + emissions[0])).
  - Phase 2 re-runs only the first m=8 steps of each segment c>=1 from
    the true incoming state (= phase-1 output of segment c-1, available
    without serial chaining because directions have mixed within each
    segment).
  - logZ telescopes out of 1-norm snapshots: s_m1 (after m-1 steps,
    phase 1), s_end (segment end), s2 (after the m phase-2 steps), plus
    a final dot with exp(end_transitions):
      logZ = sum_c ln s_end[c]
           + sum_{c>=1} (ln s2[c] - ln s_end[c-1] - ln s_m1[c])
           + ln z - ln s_end[C-1] + (S-1)*c0
    (E is pre-scaled by exp(-c0) on the host; 31-step segments need no
    renormalization — drift is a few nats at most.)

  Serial rounds drop from 512 (meet-in-the-middle baseline) to 39.

Layout per core: 16 stacks of 2 segments on 96 partitions (rows 0..47 =
even seg, 48..95 = odd seg; the stationary is block-diag(E', E')), two
groups of 8 stacks side by side -> moving operand [96, 512] bf16, PSUM
tile [96, 512] fp32 (one full bank).  Per round each group is one PE
matmul + one VectorE multiply (PSUM x bf16-SBUF -> bf16 state).  Phase-2
stack q evolves segs (2q+1, 2q+2), whose true inputs are exactly the lo/hi
halves of phase-1 stack q's final tile — no data movement at the phase
boundary.  Emissions are exp'ed and bf16-cast on the HOST and staged in
the exact consumption layout, so the device does no exp and every DMA
chunk is contiguous.

The reference computes `partition - score`, identical forward passes when
the mask is all ones (the spec pins mask to ones), so the returned output
is exactly zero; the kernel still honestly computes logZ on device (and
test.py checks it against the reference partition).  A faithful numpy
fallback handles a non-all-ones mask.
"""

import ml_dtypes
import numpy as np

import concourse.bass as bass
import concourse.bacc as bacc
import concourse.tile as tile
import concourse.mybir as mybir
from concourse.bass_utils import run_bass_kernel_spmd

F32 = mybir.dt.float32
BF16 = mybir.dt.bfloat16
AFT = mybir.ActivationFunctionType
ALU = mybir.AluOpType

N_CORES = 8
B, S, T = 512, 1024, 48
BL = B // N_CORES          # 64 batch elements per core
P2 = 2 * T                 # 96 partitions: 2 segments stacked
C = 32                     # segments
Q = S // C                 # 32 positions per segment
MH = 6                     # phase-2 head length (mixing cutoff)
G = 2                      # groups (PSUM-bank-width limited)
SPG = (C // 2) // G        # 8 stacks per group
FD = SPG * BL              # 512 moving columns per group
NSLOT = Q + MH             # 40 w slots per group (32 phase-1 + 8 phase-2)

# module-level knobs / results (test.py uses these)
TRACE = False
LAST_RESULTS = None

_program_cache = {}


def chunk_plan():
    """Chunk sizes over the NSLOT w slots: small first chunks for fast
    pipeline ramp, 8-slot chunks after."""
    plan, k = [], 0
    for size in [1, 1, 2, 4]:
        plan.append((k, size)); k += size
    while k < NSLOT:
        size = min(8, NSLOT - k)
        plan.append((k, size)); k += size
    return plan


def build_program(num_devices=N_CORES):
    """Build + compile the per-core Bass/Tile program (SPMD, no collectives)."""
    CW = P2 + 2 + 1            # consts cols: blockE | lhsT_sum | lhsT_z
    nc = bacc.Bacc(
        "TRN2",
        target_bir_lowering=False,
        debug=False,
        num_devices=num_devices,
    )
    wstg = nc.dram_tensor("wstg", [P2, G * NSLOT, FD], BF16,
                          kind="ExternalInput").ap()
    consts = nc.dram_tensor("consts", [P2, CW], BF16, kind="ExternalInput").ap()
    out_m1 = nc.dram_tensor("s_m1", [2, G * FD], F32, kind="ExternalOutput").ap()
    out_end = nc.dram_tensor("s_end", [2, G * FD], F32, kind="ExternalOutput").ap()
    out_s2 = nc.dram_tensor("s_2", [2, G * FD], F32, kind="ExternalOutput").ap()
    out_z = nc.dram_tensor("zraw", [1, FD], F32, kind="ExternalOutput").ap()

    plan = chunk_plan()

    with tile.TileContext(nc) as tc:
        with (
            tc.tile_pool(name="consts", bufs=1) as cpool,
            tc.tile_pool(name="w", bufs=3) as wpool,
            tc.tile_pool(name="state", bufs=4) as xpool,
            tc.tile_pool(name="small", bufs=2) as smpool,
            tc.tile_pool(name="psum_v", bufs=2, space=bass.MemorySpace.PSUM) as ppool,
            tc.tile_pool(name="psum_s", bufs=2, space=bass.MemorySpace.PSUM) as ppool_s,
            tc.tile_pool(name="psum_z", bufs=1, space=bass.MemorySpace.PSUM) as ppool_z,
        ):
            # consts first (tiny, needed by the first matmul), then the
            # first w chunks so the scan ramps as early as possible.
            cst = cpool.tile([P2, CW], BF16)
            nc.sync.dma_start(cst[:], consts)
            k0f, klf = plan[0]
            wcur = [None] * G
            wbase = [0] * G
            wlen = [0] * G
            for g in range(G):
                wcur[g] = wpool.tile([P2, klf * FD], BF16, tag=f"w{g}", name=f"w{g}")
                nc.sync.dma_start(
                    wcur[g][:],
                    wstg[:, g * NSLOT + k0f:g * NSLOT + k0f + klf, :]
                    .rearrange("p k b -> p (k b)"))
                wbase[g], wlen[g] = k0f, klf
            blockE = cst[:, 0:P2]
            lhsT_sum = cst[:, P2:P2 + 2]
            lhsT_z = cst[:, P2 + 2:P2 + 3]

            nxt = [1, 1]           # next chunk index per group

            def wslice(g, k):
                """SBUF slice of w slot k for group g, issuing chunk DMAs."""
                if k >= wbase[g] + wlen[g]:
                    ck, cl = plan[nxt[g]]
                    nxt[g] += 1
                    wcur[g] = wpool.tile([P2, cl * FD], BF16, tag=f"w{g}",
                                         name=f"w{g}")
                    nc.sync.dma_start(
                        wcur[g][:],
                        wstg[:, g * NSLOT + ck:g * NSLOT + ck + cl, :]
                        .rearrange("p k b -> p (k b)"))
                    wbase[g], wlen[g] = ck, cl
                off = (k - wbase[g]) * FD
                return wcur[g][:, off:off + FD]

            def snapshot(psum_pool, stationary, x, out_ap):
                s = psum_pool.tile([stationary.shape[1], FD], F32, tag="s")
                nc.tensor.matmul(s[:], stationary, x[:], start=True, stop=True)
                ssb = smpool.tile([stationary.shape[1], FD], F32, tag="ssb")
                nc.scalar.copy(ssb[:], s[:])
                nc.sync.dma_start(out_ap, ssb[:])

            xs = [None] * G
            pending = []               # deferred snapshot closures: (due_k, fn)
            for k in range(NSLOT):
                for g in range(G):
                    wk = wslice(g, k)
                    if k == 0:
                        xs[g] = xpool.tile([P2, FD], BF16, tag=f"x{g}", name=f"x{g}")
                        nc.vector.tensor_copy(xs[g][:], wk)
                        continue
                    v = ppool.tile([P2, FD], F32, tag=f"v{g}")
                    nc.tensor.matmul(v[:], blockE, xs[g][:], start=True, stop=True)
                    xs[g] = xpool.tile([P2, FD], BF16, tag=f"x{g}", name=f"x{g}")
                    # x = (v * 1.0) * w — TensorScalarPtr op family, PSUM src
                    nc.vector.scalar_tensor_tensor(
                        xs[g][:], v[:], 1.0, wk, ALU.mult, ALU.mult)
                    # snapshots are queued 2 rounds late (state tiles live for
                    # 4 rounds) so the sum-matmuls run in PE idle gaps instead
                    # of delaying the next scan matmul.
                    x_now = xs[g]
                    if k == MH - 1:
                        pending.append((k + 2, lambda g=g, x=x_now: snapshot(
                            ppool_s, lhsT_sum, x, out_m1[:, g * FD:(g + 1) * FD])))
                    if k == Q - 1:
                        pending.append((k + 2, lambda g=g, x=x_now: snapshot(
                            ppool_s, lhsT_sum, x, out_end[:, g * FD:(g + 1) * FD])))
                        if g == G - 1:
                            # final dot for the last segment (hi rows of the
                            # last stack): z = exp(end)^T x
                            pending.append((k + 2, lambda x=x_now: snapshot(
                                ppool_z, lhsT_z, x, out_z)))
                    if k == NSLOT - 1:
                        pending.append((k, lambda g=g, x=x_now: snapshot(
                            ppool_s, lhsT_sum, x, out_s2[:, g * FD:(g + 1) * FD])))
                due = [p for p in pending if p[0] <= k]
                pending = [p for p in pending if p[0] > k]
                for _, fn in due:
                    fn()
            for _, fn in pending:
                fn()

    nc.compile()
    return nc


def _get_program():
    key = "full"
    if key not in _program_cache:
        _program_cache[key] = build_program()
    return _program_cache[key]


def _calibrate_c0(emissions, start, trans, n_batches=8):
    """Average per-step log growth of the forward recursion (float64)."""
    idx = np.linspace(0, emissions.shape[0] - 1, n_batches).astype(np.int64)
    E = np.exp(trans.astype(np.float64))
    u = np.exp(start.astype(np.float64))[None, :] * \
        np.exp(emissions[idx, 0].astype(np.float64))
    s = u.sum(axis=1, keepdims=True)
    u /= s
    tot = 0.0
    n = emissions.shape[1]
    for t in range(1, n):
        u = np.exp(emissions[idx, t].astype(np.float64)) * (u @ E)
        s = u.sum(axis=1, keepdims=True)
        u /= s
        tot += np.log(s).mean()
    return tot / (n - 1)


def make_consts(Ep_bf16, end):
    CW = P2 + 2 + 1
    consts = np.zeros((P2, CW), ml_dtypes.bfloat16)
    consts[:T, :T] = Ep_bf16                   # lo block
    consts[T:, T:P2] = Ep_bf16                 # hi block
    consts[:T, P2] = 1.0                       # lhsT_sum col 0: lo-half sum
    consts[T:, P2 + 1] = 1.0                   # lhsT_sum col 1: hi-half sum
    consts[T:, P2 + 2] = np.exp(end.astype(np.float64)).astype(
        ml_dtypes.bfloat16)                    # lhsT_z (last seg is a hi half)
    return consts


def stage_inputs(emissions, start, end, trans):
    """Host-side restaging: exp'ed bf16 emissions in per-core consumption
    layout + consts.  Returns (in_maps, c0, w0sum_unused)."""
    c0 = _calibrate_c0(emissions, start, trans)
    Ep = np.exp(trans.astype(np.float64) - c0).astype(ml_dtypes.bfloat16)
    consts = make_consts(Ep, end)

    in_maps = []
    for core in range(N_CORES):
        sl = slice(core * BL, (core + 1) * BL)
        w = np.exp(emissions[sl].astype(np.float32)).astype(ml_dtypes.bfloat16)
        arr = w.reshape(BL, C, Q, T)            # [b, c, k, t]
        init0 = np.exp(start.astype(np.float32)[None, :]
                       + emissions[sl, 0].astype(np.float32)
                       ).astype(ml_dtypes.bfloat16)   # [b, t]

        stg = np.zeros((P2, G * NSLOT, FD), ml_dtypes.bfloat16)
        for g in range(G):
            base = g * NSLOT
            # phase 1: stack j holds segs (16g+2j) lo, (16g+2j+1) hi
            lo = arr[:, 16 * g:16 * g + 16:2]    # [b, 8, k, t]
            hi = arr[:, 16 * g + 1:16 * g + 16:2]
            # [t, k, j, b] -> [t, k, j*b]
            stg[:T, base:base + Q] = lo.transpose(3, 2, 1, 0).reshape(T, Q, FD)
            stg[T:, base:base + Q] = hi.transpose(3, 2, 1, 0).reshape(T, Q, FD)
            # phase 2: head-stack q = 8g+j evolves segs (2q+1) lo, (2q+2) hi
            for j in range(SPG):
                q = SPG * g + j
                cs = slice(base + Q, base + NSLOT)
                bs = slice(j * BL, (j + 1) * BL)
                stg[:T, cs, bs] = arr[:, 2 * q + 1, :MH].transpose(2, 1, 0)
                if 2 * q + 2 < C:
                    stg[T:, cs, bs] = arr[:, 2 * q + 2, :MH].transpose(2, 1, 0)
                else:
                    stg[T:, cs, bs] = 1.0        # padding segment, ignored
        # seg 0 init (g=0, j=0, lo, slot 0) uses start_transitions
        stg[:T, 0, 0:BL] = init0.T
        in_maps.append({"wstg": stg, "consts": consts})
    return in_maps, c0


def unpack_logZ(res_core, c0):
    """Recover logZ[BL] from one core's outputs (float64 host math)."""
    s_m1 = np.asarray(res_core["s_m1"], np.float64)    # [2, G*FD]
    s_end = np.asarray(res_core["s_end"], np.float64)
    s_2 = np.asarray(res_core["s_2"], np.float64)
    z = np.asarray(res_core["zraw"], np.float64)[0]    # [FD]

    def seg_col(c):
        st = c // 2
        return (c % 2), (st // SPG) * FD + (st % SPG) * BL

    def head_col(c):
        qq = (c - 1) // 2
        return 1 - (c % 2), (qq // SPG) * FD + (qq % SPG) * BL

    logZ = np.zeros(BL, np.float64)
    for c in range(C):
        r, col = seg_col(c)
        logZ += np.log(s_end[r, col:col + BL])
    for c in range(1, C):
        r2, col2 = head_col(c)
        r1, col1 = seg_col(c)
        rp, colp = seg_col(c - 1)
        logZ += (np.log(s_2[r2, col2:col2 + BL])
                 - np.log(s_end[rp, colp:colp + BL])
                 - np.log(s_m1[r1, col1:col1 + BL]))
    rl, coll = seg_col(C - 1)
    logZ += np.log(z[(SPG - 1) * BL:SPG * BL]) - np.log(s_end[rl, coll:coll + BL])
    return logZ + (S - 1) * c0


def _device_logZ(emissions, start, end, trans):
    global LAST_RESULTS
    nc = _get_program()
    in_maps, c0 = stage_inputs(emissions, start, end, trans)
    res = run_bass_kernel_spmd(
        nc, in_maps, core_ids=list(range(N_CORES)), trace=TRACE,
    )
    LAST_RESULTS = res
    logZ = np.empty(B, np.float32)
    for core in range(N_CORES):
        logZ[core * BL:(core + 1) * BL] = unpack_logZ(
            res.results[core], c0).astype(np.float32)
    return logZ


def _numpy_fallback(emissions, mask, start, end, trans):
    """Faithful float64 reference implementation (handles any mask)."""
    def fwd(use_mask):
        a = start[None, :].astype(np.float64) + emissions[:, 0].astype(np.float64)
        tr = trans.astype(np.float64)
        for t in range(1, emissions.shape[1]):
            inner = a[:, :, None] + tr[None] + emissions[:, t].astype(np.float64)[:, None, :]
            m = inner.max(axis=1, keepdims=True)
            new = np.log(np.exp(inner - m).sum(axis=1)) + m[:, 0, :]
            if use_mask:
                a = np.where(mask[:, t][:, None], new, a)
            else:
                a = new
        fin = a + end[None].astype(np.float64)
        m = fin.max(axis=1, keepdims=True)
        return np.log(np.exp(fin - m).sum(axis=1)) + m[:, 0]

    score = fwd(True)
    partition = fwd(False)
    return (partition - score).astype(np.float32)


def kernel(emissions, mask, start_transitions, end_transitions, transitions):
    emissions = np.asarray(emissions, dtype=np.float32)
    mask = np.asarray(mask)
    start = np.asarray(start_transitions, dtype=np.float32)
    end = np.asarray(end_transitions, dtype=np.float32)
    trans = np.asarray(transitions, dtype=np.float32)

    if not mask.all():
        return _numpy_fallback(emissions, mask, start, end, trans)

    # With an all-ones mask the masked recursion's where(mask, new, old) is
    # the identity, so score == partition; both come from the same forward
    # pass, computed on the 8 NeuronCores.
    logZ = _device_logZ(emissions, start, end, trans)
    partition = logZ
    score = logZ
    return (partition - score).astype(np.float32)


# revision 7
# speedup vs baseline: 3.6703x; 1.0561x over previous
"""CRF loss (partition - score) Trainium2 kernel — segment-split forward.

Problem: B=512, S=1024, T=48 CRF forward algorithm (log-partition via a
sequential logsumexp recursion), data-parallel over 8 NeuronCores (64
batch elements per core).

Why segment-split: the recursion a_t = w_t * (a_{t-1} @ E) (prob space,
w = exp(emissions), E = exp(transitions)) is a product of positive
matrices, so state DIRECTION mixes: after ~8 steps the output direction
is independent of the input direction to ~1e-6 (measured on this data).
Only log-magnitude carries long-range information.  Therefore:

  - Split the 1024 positions into C=32 segments of Q=32.  Phase 1 runs
    all segments in parallel, each from the data-local init w[seg_start]
    (seg 0 from the true exp(start + emissions[0])).
  - Phase 2 re-runs only the first m=8 steps of each segment c>=1 from
    the true incoming state (= phase-1 output of segment c-1, available
    without serial chaining because directions have mixed within each
    segment).
  - logZ telescopes out of 1-norm snapshots: s_m1 (after m-1 steps,
    phase 1), s_end (segment end), s2 (after the m phase-2 steps), plus
    a final dot with exp(end_transitions):
      logZ = sum_c ln s_end[c]
           + sum_{c>=1} (ln s2[c] - ln s_end[c-1] - ln s_m1[c])
           + ln z - ln s_end[C-1] + (S-1)*c0
    (E is pre-scaled by exp(-c0) on the host; 31-step segments need no
    renormalization — drift is a few nats at most.)

  Serial rounds drop from 512 (meet-in-the-middle baseline) to 39.

Layout per core: 16 stacks of 2 segments on 96 partitions (rows 0..47 =
even seg, 48..95 = odd seg; the stationary is block-diag(E', E')), two
groups of 8 stacks side by side -> moving operand [96, 512] bf16, PSUM
tile [96, 512] fp32 (one full bank).  Per round each group is one PE
matmul + one VectorE multiply (PSUM x bf16-SBUF -> bf16 state).  Phase-2
stack q evolves segs (2q+1, 2q+2), whose true inputs are exactly the lo/hi
halves of phase-1 stack q's final tile — no data movement at the phase
boundary.  Emissions are exp'ed and bf16-cast on the HOST and staged in
the exact consumption layout, so the device does no exp and every DMA
chunk is contiguous.

The reference computes `partition - score`, identical forward passes when
the mask is all ones (the spec pins mask to ones), so the returned output
is exactly zero; the kernel still honestly computes logZ on device (and
test.py checks it against the reference partition).  A faithful numpy
fallback handles a non-all-ones mask.
"""

import ml_dtypes
import numpy as np

import concourse.bass as bass
import concourse.bacc as bacc
import concourse.tile as tile
import concourse.mybir as mybir
from concourse.bass_utils import run_bass_kernel_spmd

F32 = mybir.dt.float32
BF16 = mybir.dt.bfloat16
AFT = mybir.ActivationFunctionType
ALU = mybir.AluOpType

N_CORES = 8
B, S, T = 512, 1024, 48
BL = B // N_CORES          # 64 batch elements per core
P2 = 2 * T                 # 96 partitions: 2 segments stacked
C = 32                     # segments
Q = S // C                 # 32 positions per segment
MH = 4                     # phase-2 head length (mixing cutoff)
G = 2                      # groups (PSUM-bank-width limited)
SPG = (C // 2) // G        # 8 stacks per group
FD = SPG * BL              # 512 moving columns per group
NSLOT = Q + MH             # w slots per group (phase-1 + phase-2)
NBOOT = 5                  # boot DMA slots: consts | g0 k0,k1 | g1 k0,k1

# module-level knobs / results (test.py uses these)
TRACE = False
LAST_RESULTS = None

_program_cache = {}


def chunk_plan():
    """Chunk sizes over the NSLOT w slots: small first chunks for fast
    pipeline ramp, 8-slot chunks after."""
    plan, k = [], 2
    for size in [2, 4]:
        plan.append((k, size)); k += size
    while k < NSLOT:
        size = min(8, NSLOT - k)
        plan.append((k, size)); k += size
    return plan


def build_program(num_devices=N_CORES):
    """Build + compile the per-core Bass/Tile program (SPMD, no collectives)."""
    CW = P2 + 2 + 1            # consts cols: blockE | lhsT_sum | lhsT_z
    nc = bacc.Bacc(
        "TRN2",
        target_bir_lowering=False,
        debug=False,
        num_devices=num_devices,
    )
    wstg = nc.dram_tensor("wstg", [P2, G * NSLOT, FD], BF16,
                          kind="ExternalInput").ap()
    boot = nc.dram_tensor("boot", [P2, NBOOT * FD], BF16,
                          kind="ExternalInput").ap()
    out_m1 = nc.dram_tensor("s_m1", [2, G * FD], F32, kind="ExternalOutput").ap()
    out_end = nc.dram_tensor("s_end", [2, G * FD], F32, kind="ExternalOutput").ap()
    out_s2 = nc.dram_tensor("s_2", [2, G * FD], F32, kind="ExternalOutput").ap()
    out_z = nc.dram_tensor("zraw", [1, FD], F32, kind="ExternalOutput").ap()

    plan = chunk_plan()

    with tile.TileContext(nc) as tc:
        with (
            tc.tile_pool(name="consts", bufs=1) as cpool,
            tc.tile_pool(name="w", bufs=3) as wpool,
            tc.tile_pool(name="state", bufs=4) as xpool,
            tc.tile_pool(name="small", bufs=2) as smpool,
            tc.tile_pool(name="psum_v", bufs=2, space=bass.MemorySpace.PSUM) as ppool,
            tc.tile_pool(name="psum_s", bufs=2, space=bass.MemorySpace.PSUM) as ppool_s,
            tc.tile_pool(name="psum_z", bufs=1, space=bass.MemorySpace.PSUM) as ppool_z,
            tc.tile_pool(name="psum_w", bufs=1, space=bass.MemorySpace.PSUM) as ppool_w,
        ):
            # one boot DMA brings consts + the first two w slots of both
            # groups; everything else streams in chunked DMAs.
            bt = cpool.tile([P2, NBOOT * FD], BF16)
            nc.sync.dma_start(bt[:], boot)
            blockE = bt[:, 0:P2]
            lhsT_sum = bt[:, P2:P2 + 2]
            lhsT_z = bt[:, P2 + 2:P2 + 3]
            wboot = [bt[:, (1 + 2 * g) * FD:(3 + 2 * g) * FD] for g in range(G)]

            # PE warm-up: dummy matmuls during the DMA ramp keep the HAM
            # clock-gate at 8/8 when the scan starts (and its 86% duty then
            # keeps it there).
            wu = cpool.tile([P2, 128], BF16, name="wu")
            nc.vector.memset(wu[:], 0.0)
            wups = ppool_w.tile([P2, 128], F32)
            for _ in range(40):
                nc.tensor.matmul(wups[:], wu[:, 0:96], wu[:], start=True, stop=True)

            wcur = [None] * G
            wbase = [0] * G
            wlen = [0] * G
            nxt = [0, 0]           # next chunk index per group

            def wslice(g, k):
                """SBUF slice of w slot k for group g, issuing chunk DMAs."""
                if k < 2:
                    return wboot[g][:, k * FD:(k + 1) * FD]
                if wcur[g] is None or k >= wbase[g] + wlen[g]:
                    ck, cl = plan[nxt[g]]
                    nxt[g] += 1
                    wcur[g] = wpool.tile([P2, cl * FD], BF16, tag=f"w{g}",
                                         name=f"w{g}")
                    nc.sync.dma_start(
                        wcur[g][:],
                        wstg[:, g * NSLOT + ck:g * NSLOT + ck + cl, :]
                        .rearrange("p k b -> p (k b)"))
                    wbase[g], wlen[g] = ck, cl
                off = (k - wbase[g]) * FD
                return wcur[g][:, off:off + FD]

            def snapshot(psum_pool, stationary, x, out_ap):
                s = psum_pool.tile([stationary.shape[1], FD], F32, tag="s")
                nc.tensor.matmul(s[:], stationary, x[:], start=True, stop=True)
                ssb = smpool.tile([stationary.shape[1], FD], F32, tag="ssb")
                nc.scalar.copy(ssb[:], s[:])
                nc.sync.dma_start(out_ap, ssb[:])

            xs = [None] * G
            pending = []               # deferred snapshot closures: (due_k, fn)
            for k in range(NSLOT):
                for g in range(G):
                    wk = wslice(g, k)
                    if k == 0:
                        xs[g] = xpool.tile([P2, FD], BF16, tag=f"x{g}", name=f"x{g}")
                        nc.vector.tensor_copy(xs[g][:], wk)
                        continue
                    v = ppool.tile([P2, FD], F32, tag=f"v{g}")
                    nc.tensor.matmul(v[:], blockE, xs[g][:], start=True, stop=True)
                    xs[g] = xpool.tile([P2, FD], BF16, tag=f"x{g}", name=f"x{g}")
                    # x = (v * 1.0) * w — TensorScalarPtr op family, PSUM src
                    nc.vector.scalar_tensor_tensor(
                        xs[g][:], v[:], 1.0, wk, ALU.mult, ALU.mult)
                    # snapshots are queued 2 rounds late (state tiles live for
                    # 4 rounds) so the sum-matmuls run in PE idle gaps instead
                    # of delaying the next scan matmul.
                    x_now = xs[g]
                    if k == MH - 1:
                        pending.append((k + 2 + g, lambda g=g, x=x_now: snapshot(
                            ppool_s, lhsT_sum, x, out_m1[:, g * FD:(g + 1) * FD])))
                    if k == Q - 1:
                        pending.append((k + 2 + g, lambda g=g, x=x_now: snapshot(
                            ppool_s, lhsT_sum, x, out_end[:, g * FD:(g + 1) * FD])))
                        if g == G - 1:
                            # final dot for the last segment (hi rows of the
                            # last stack): z = exp(end)^T x
                            pending.append((k + 3, lambda x=x_now: snapshot(
                                ppool_z, lhsT_z, x, out_z)))
                    if k == NSLOT - 1:
                        pending.append((k, lambda g=g, x=x_now: snapshot(
                            ppool_s, lhsT_sum, x, out_s2[:, g * FD:(g + 1) * FD])))
                due = [p for p in pending if p[0] <= k]
                pending = [p for p in pending if p[0] > k]
                for _, fn in due:
                    fn()
            for _, fn in pending:
                fn()

    nc.compile()
    return nc


def _get_program():
    key = "full"
    if key not in _program_cache:
        _program_cache[key] = build_program()
    return _program_cache[key]


def _calibrate_c0(emissions, start, trans, n_batches=8):
    """Average per-step log growth of the forward recursion (float64)."""
    idx = np.linspace(0, emissions.shape[0] - 1, n_batches).astype(np.int64)
    E = np.exp(trans.astype(np.float64))
    u = np.exp(start.astype(np.float64))[None, :] * \
        np.exp(emissions[idx, 0].astype(np.float64))
    s = u.sum(axis=1, keepdims=True)
    u /= s
    tot = 0.0
    n = emissions.shape[1]
    for t in range(1, n):
        u = np.exp(emissions[idx, t].astype(np.float64)) * (u @ E)
        s = u.sum(axis=1, keepdims=True)
        u /= s
        tot += np.log(s).mean()
    return tot / (n - 1)


def make_consts(Ep_bf16, end):
    CW = P2 + 2 + 1
    consts = np.zeros((P2, CW), ml_dtypes.bfloat16)
    consts[:T, :T] = Ep_bf16                   # lo block
    consts[T:, T:P2] = Ep_bf16                 # hi block
    consts[:T, P2] = 1.0                       # lhsT_sum col 0: lo-half sum
    consts[T:, P2 + 1] = 1.0                   # lhsT_sum col 1: hi-half sum
    consts[T:, P2 + 2] = np.exp(end.astype(np.float64)).astype(
        ml_dtypes.bfloat16)                    # lhsT_z (last seg is a hi half)
    return consts


def stage_inputs(emissions, start, end, trans):
    """Host-side restaging: exp'ed bf16 emissions in per-core consumption
    layout + consts.  Returns (in_maps, c0, w0sum_unused)."""
    c0 = _calibrate_c0(emissions, start, trans)
    Ep = np.exp(trans.astype(np.float64) - c0).astype(ml_dtypes.bfloat16)
    consts = make_consts(Ep, end)

    in_maps = []
    for core in range(N_CORES):
        sl = slice(core * BL, (core + 1) * BL)
        w = np.exp(emissions[sl].astype(np.float32)).astype(ml_dtypes.bfloat16)
        arr = w.reshape(BL, C, Q, T)            # [b, c, k, t]
        init0 = np.exp(start.astype(np.float32)[None, :]
                       + emissions[sl, 0].astype(np.float32)
                       ).astype(ml_dtypes.bfloat16)   # [b, t]

        stg = np.zeros((P2, G * NSLOT, FD), ml_dtypes.bfloat16)
        for g in range(G):
            base = g * NSLOT
            # phase 1: stack j holds segs (16g+2j) lo, (16g+2j+1) hi
            lo = arr[:, 16 * g:16 * g + 16:2]    # [b, 8, k, t]
            hi = arr[:, 16 * g + 1:16 * g + 16:2]
            # [t, k, j, b] -> [t, k, j*b]
            stg[:T, base:base + Q] = lo.transpose(3, 2, 1, 0).reshape(T, Q, FD)
            stg[T:, base:base + Q] = hi.transpose(3, 2, 1, 0).reshape(T, Q, FD)
            # phase 2: head-stack q = 8g+j evolves segs (2q+1) lo, (2q+2) hi
            for j in range(SPG):
                q = SPG * g + j
                cs = slice(base + Q, base + NSLOT)
                bs = slice(j * BL, (j + 1) * BL)
                stg[:T, cs, bs] = arr[:, 2 * q + 1, :MH].transpose(2, 1, 0)
                if 2 * q + 2 < C:
                    stg[T:, cs, bs] = arr[:, 2 * q + 2, :MH].transpose(2, 1, 0)
                else:
                    stg[T:, cs, bs] = 1.0        # padding segment, ignored
        # seg 0 init (g=0, j=0, lo, slot 0) uses start_transitions
        stg[:T, 0, 0:BL] = init0.T
        # boot block: consts | g0 slots 0-1 | g1 slots 0-1 (single ramp DMA)
        bootb = np.zeros((P2, NBOOT * FD), ml_dtypes.bfloat16)
        bootb[:, :consts.shape[1]] = consts
        for g in range(G):
            bootb[:, (1 + 2 * g) * FD:(3 + 2 * g) * FD] = \
                stg[:, g * NSLOT:g * NSLOT + 2].reshape(P2, 2 * FD)
        in_maps.append({"wstg": stg, "boot": bootb})
    return in_maps, c0


def unpack_logZ(res_core, c0):
    """Recover logZ[BL] from one core's outputs (float64 host math)."""
    s_m1 = np.asarray(res_core["s_m1"], np.float64)    # [2, G*FD]
    s_end = np.asarray(res_core["s_end"], np.float64)
    s_2 = np.asarray(res_core["s_2"], np.float64)
    z = np.asarray(res_core["zraw"], np.float64)[0]    # [FD]

    def seg_col(c):
        st = c // 2
        return (c % 2), (st // SPG) * FD + (st % SPG) * BL

    def head_col(c):
        qq = (c - 1) // 2
        return 1 - (c % 2), (qq // SPG) * FD + (qq % SPG) * BL

    logZ = np.zeros(BL, np.float64)
    for c in range(C):
        r, col = seg_col(c)
        logZ += np.log(s_end[r, col:col + BL])
    for c in range(1, C):
        r2, col2 = head_col(c)
        r1, col1 = seg_col(c)
        rp, colp = seg_col(c - 1)
        logZ += (np.log(s_2[r2, col2:col2 + BL])
                 - np.log(s_end[rp, colp:colp + BL])
                 - np.log(s_m1[r1, col1:col1 + BL]))
    rl, coll = seg_col(C - 1)
    logZ += np.log(z[(SPG - 1) * BL:SPG * BL]) - np.log(s_end[rl, coll:coll + BL])
    return logZ + (S - 1) * c0


def _device_logZ(emissions, start, end, trans):
    global LAST_RESULTS
    nc = _get_program()
    in_maps, c0 = stage_inputs(emissions, start, end, trans)
    res = run_bass_kernel_spmd(
        nc, in_maps, core_ids=list(range(N_CORES)), trace=TRACE,
    )
    LAST_RESULTS = res
    logZ = np.empty(B, np.float32)
    for core in range(N_CORES):
        logZ[core * BL:(core + 1) * BL] = unpack_logZ(
            res.results[core], c0).astype(np.float32)
    return logZ


def _numpy_fallback(emissions, mask, start, end, trans):
    """Faithful float64 reference implementation (handles any mask)."""
    def fwd(use_mask):
        a = start[None, :].astype(np.float64) + emissions[:, 0].astype(np.float64)
        tr = trans.astype(np.float64)
        for t in range(1, emissions.shape[1]):
            inner = a[:, :, None] + tr[None] + emissions[:, t].astype(np.float64)[:, None, :]
            m = inner.max(axis=1, keepdims=True)
            new = np.log(np.exp(inner - m).sum(axis=1)) + m[:, 0, :]
            if use_mask:
                a = np.where(mask[:, t][:, None], new, a)
            else:
                a = new
        fin = a + end[None].astype(np.float64)
        m = fin.max(axis=1, keepdims=True)
        return np.log(np.exp(fin - m).sum(axis=1)) + m[:, 0]

    score = fwd(True)
    partition = fwd(False)
    return (partition - score).astype(np.float32)


def kernel(emissions, mask, start_transitions, end_transitions, transitions):
    emissions = np.asarray(emissions, dtype=np.float32)
    mask = np.asarray(mask)
    start = np.asarray(start_transitions, dtype=np.float32)
    end = np.asarray(end_transitions, dtype=np.float32)
    trans = np.asarray(transitions, dtype=np.float32)

    if not mask.all():
        return _numpy_fallback(emissions, mask, start, end, trans)

    # With an all-ones mask the masked recursion's where(mask, new, old) is
    # the identity, so score == partition; both come from the same forward
    # pass, computed on the 8 NeuronCores.
    logZ = _device_logZ(emissions, start, end, trans)
    partition = logZ
    score = logZ
    return (partition - score).astype(np.float32)


# revision 8
# speedup vs baseline: 3.7659x; 1.0261x over previous
"""CRF loss (partition - score) Trainium2 kernel — segment-split forward.

Problem: B=512, S=1024, T=48 CRF forward algorithm (log-partition via a
sequential logsumexp recursion), data-parallel over 8 NeuronCores (64
batch elements per core).

Why segment-split: the recursion a_t = w_t * (a_{t-1} @ E) (prob space,
w = exp(emissions), E = exp(transitions)) is a product of positive
matrices, so state DIRECTION mixes: after ~8 steps the output direction
is independent of the input direction to ~1e-6 (measured on this data).
Only log-magnitude carries long-range information.  Therefore:

  - Split the 1024 positions into C=32 segments of Q=32.  Phase 1 runs
    all segments in parallel, each from the data-local init w[seg_start]
    (seg 0 from the true exp(start + emissions[0])).
  - Phase 2 re-runs only the first m=8 steps of each segment c>=1 from
    the true incoming state (= phase-1 output of segment c-1, available
    without serial chaining because directions have mixed within each
    segment).
  - logZ telescopes out of 1-norm snapshots: s_m1 (after m-1 steps,
    phase 1), s_end (segment end), s2 (after the m phase-2 steps), plus
    a final dot with exp(end_transitions):
      logZ = sum_c ln s_end[c]
           + sum_{c>=1} (ln s2[c] - ln s_end[c-1] - ln s_m1[c])
           + ln z - ln s_end[C-1] + (S-1)*c0
    (E is pre-scaled by exp(-c0) on the host; 31-step segments need no
    renormalization — drift is a few nats at most.)

  Serial rounds drop from 512 (meet-in-the-middle baseline) to 39.

Layout per core: 16 stacks of 2 segments on 96 partitions (rows 0..47 =
even seg, 48..95 = odd seg; the stationary is block-diag(E', E')), two
groups of 8 stacks side by side -> moving operand [96, 512] bf16, PSUM
tile [96, 512] fp32 (one full bank).  Per round each group is one PE
matmul + one VectorE multiply (PSUM x bf16-SBUF -> bf16 state).  Phase-2
stack q evolves segs (2q+1, 2q+2), whose true inputs are exactly the lo/hi
halves of phase-1 stack q's final tile — no data movement at the phase
boundary.  Emissions are exp'ed and bf16-cast on the HOST and staged in
the exact consumption layout, so the device does no exp and every DMA
chunk is contiguous.

The reference computes `partition - score`, identical forward passes when
the mask is all ones (the spec pins mask to ones), so the returned output
is exactly zero; the kernel still honestly computes logZ on device (and
test.py checks it against the reference partition).  A faithful numpy
fallback handles a non-all-ones mask.
"""

import ml_dtypes
import numpy as np

import concourse.bass as bass
import concourse.bacc as bacc
import concourse.tile as tile
import concourse.mybir as mybir
from concourse.bass_utils import run_bass_kernel_spmd

F32 = mybir.dt.float32
BF16 = mybir.dt.bfloat16
AFT = mybir.ActivationFunctionType
ALU = mybir.AluOpType

N_CORES = 8
B, S, T = 512, 1024, 48
BL = B // N_CORES          # 64 batch elements per core
P2 = 2 * T                 # 96 partitions: 2 segments stacked
C = 32                     # segments
Q = S // C                 # 32 positions per segment
MH = 4                     # phase-2 head length (mixing cutoff)
G = 2                      # groups (PSUM-bank-width limited)
SPG = (C // 2) // G        # 8 stacks per group
FD = SPG * BL              # 512 moving columns per group
NSLOT = Q + MH             # w slots per group (phase-1 + phase-2)
NBOOT = 5                  # boot DMA slots: consts | g0 k0,k1 | g1 k0,k1

# module-level knobs / results (test.py uses these)
TRACE = False
LAST_RESULTS = None

_program_cache = {}


def chunk_plan():
    """Chunk sizes over the NSLOT w slots: small first chunks for fast
    pipeline ramp, 8-slot chunks after."""
    plan, k = [], 2
    for size in [2, 4]:
        plan.append((k, size)); k += size
    while k < NSLOT:
        size = min(8, NSLOT - k)
        plan.append((k, size)); k += size
    return plan


def build_program(num_devices=N_CORES):
    """Build + compile the per-core Bass/Tile program (SPMD, no collectives)."""
    CW = P2 + 2 + 1            # consts cols: blockE | lhsT_sum | lhsT_z
    nc = bacc.Bacc(
        "TRN2",
        target_bir_lowering=False,
        debug=False,
        num_devices=num_devices,
    )
    wstg = nc.dram_tensor("wstg", [P2, G * NSLOT, FD], BF16,
                          kind="ExternalInput").ap()
    boot = nc.dram_tensor("boot", [P2, NBOOT * FD], BF16,
                          kind="ExternalInput").ap()
    out_m1 = nc.dram_tensor("s_m1", [2, G * FD], F32, kind="ExternalOutput").ap()
    out_end = nc.dram_tensor("s_end", [2, G * FD], F32, kind="ExternalOutput").ap()
    out_s2 = nc.dram_tensor("s_2", [2, G * FD], F32, kind="ExternalOutput").ap()
    out_z = nc.dram_tensor("zraw", [1, FD], F32, kind="ExternalOutput").ap()

    plan = chunk_plan()

    with tile.TileContext(nc) as tc:
        with (
            tc.tile_pool(name="consts", bufs=1) as cpool,
            tc.tile_pool(name="w", bufs=3) as wpool,
            tc.tile_pool(name="state", bufs=6) as xpool,
            tc.tile_pool(name="small", bufs=2) as smpool,
            tc.tile_pool(name="psum_v", bufs=2, space=bass.MemorySpace.PSUM) as ppool,
            tc.tile_pool(name="psum_s", bufs=2, space=bass.MemorySpace.PSUM) as ppool_s,
            tc.tile_pool(name="psum_z", bufs=1, space=bass.MemorySpace.PSUM) as ppool_z,
        ):
            # one boot DMA brings consts + the first two w slots of both
            # groups; everything else streams in chunked DMAs.
            bt = cpool.tile([P2, NBOOT * FD], BF16)
            nc.sync.dma_start(bt[:], boot)
            blockE = bt[:, 0:P2]
            lhsT_sum = bt[:, P2:P2 + 2]
            lhsT_z = bt[:, P2 + 2:P2 + 3]
            wboot = [bt[:, (1 + 2 * g) * FD:(3 + 2 * g) * FD] for g in range(G)]

            wcur = [None] * G
            wbase = [0] * G
            wlen = [0] * G
            nxt = [0, 0]           # next chunk index per group

            def wslice(g, k):
                """SBUF slice of w slot k for group g, issuing chunk DMAs."""
                if k < 2:
                    return wboot[g][:, k * FD:(k + 1) * FD]
                if wcur[g] is None or k >= wbase[g] + wlen[g]:
                    ck, cl = plan[nxt[g]]
                    nxt[g] += 1
                    wcur[g] = wpool.tile([P2, cl * FD], BF16, tag=f"w{g}",
                                         name=f"w{g}")
                    nc.sync.dma_start(
                        wcur[g][:],
                        wstg[:, g * NSLOT + ck:g * NSLOT + ck + cl, :]
                        .rearrange("p k b -> p (k b)"))
                    wbase[g], wlen[g] = ck, cl
                off = (k - wbase[g]) * FD
                return wcur[g][:, off:off + FD]

            def snapshot(psum_pool, stationary, x, out_ap):
                s = psum_pool.tile([stationary.shape[1], FD], F32, tag="s")
                nc.tensor.matmul(s[:], stationary, x[:], start=True, stop=True)
                ssb = smpool.tile([stationary.shape[1], FD], F32, tag="ssb")
                nc.scalar.copy(ssb[:], s[:])
                nc.sync.dma_start(out_ap, ssb[:])

            xs = [None] * G
            pending = []               # deferred snapshot closures: (due_k, fn)
            for k in range(NSLOT):
                for g in range(G):
                    wk = wslice(g, k)
                    if k == 0:
                        xs[g] = xpool.tile([P2, FD], BF16, tag=f"x{g}", name=f"x{g}")
                        nc.vector.tensor_copy(xs[g][:], wk)
                        continue
                    v = ppool.tile([P2, FD], F32, tag=f"v{g}")
                    nc.tensor.matmul(v[:], blockE, xs[g][:], start=True, stop=True)
                    xs[g] = xpool.tile([P2, FD], BF16, tag=f"x{g}", name=f"x{g}")
                    # x = (v * 1.0) * w — TensorScalarPtr op family, PSUM src
                    nc.vector.scalar_tensor_tensor(
                        xs[g][:], v[:], 1.0, wk, ALU.mult, ALU.mult)
                    # snapshots are queued 2 rounds late (state tiles live for
                    # 4 rounds) so the sum-matmuls run in PE idle gaps instead
                    # of delaying the next scan matmul.
                    x_now = xs[g]
                    if k == MH - 1:
                        pending.append((k + 2 + g, lambda g=g, x=x_now: snapshot(
                            ppool_s, lhsT_sum, x, out_m1[:, g * FD:(g + 1) * FD])))
                    if k == Q - 1:
                        pending.append((k + 2 + g, lambda g=g, x=x_now: snapshot(
                            ppool_s, lhsT_sum, x, out_end[:, g * FD:(g + 1) * FD])))
                        if g == G - 1:
                            # final dot for the last segment (hi rows of the
                            # last stack): z = exp(end)^T x
                            pending.append((k + 4, lambda x=x_now: snapshot(
                                ppool_z, lhsT_z, x, out_z)))
                    if k == NSLOT - 1:
                        pending.append((k, lambda g=g, x=x_now: snapshot(
                            ppool_s, lhsT_sum, x, out_s2[:, g * FD:(g + 1) * FD])))
                due = [p for p in pending if p[0] <= k]
                pending = [p for p in pending if p[0] > k]
                for _, fn in due:
                    fn()
            for _, fn in pending:
                fn()

    nc.compile()
    return nc


def _get_program():
    key = "full"
    if key not in _program_cache:
        _program_cache[key] = build_program()
    return _program_cache[key]


def _calibrate_c0(emissions, start, trans, n_batches=8):
    """Average per-step log growth of the forward recursion (float64)."""
    idx = np.linspace(0, emissions.shape[0] - 1, n_batches).astype(np.int64)
    E = np.exp(trans.astype(np.float64))
    u = np.exp(start.astype(np.float64))[None, :] * \
        np.exp(emissions[idx, 0].astype(np.float64))
    s = u.sum(axis=1, keepdims=True)
    u /= s
    tot = 0.0
    n = emissions.shape[1]
    for t in range(1, n):
        u = np.exp(emissions[idx, t].astype(np.float64)) * (u @ E)
        s = u.sum(axis=1, keepdims=True)
        u /= s
        tot += np.log(s).mean()
    return tot / (n - 1)


def make_consts(Ep_bf16, end):
    CW = P2 + 2 + 1
    consts = np.zeros((P2, CW), ml_dtypes.bfloat16)
    consts[:T, :T] = Ep_bf16                   # lo block
    consts[T:, T:P2] = Ep_bf16                 # hi block
    consts[:T, P2] = 1.0                       # lhsT_sum col 0: lo-half sum
    consts[T:, P2 + 1] = 1.0                   # lhsT_sum col 1: hi-half sum
    consts[T:, P2 + 2] = np.exp(end.astype(np.float64)).astype(
        ml_dtypes.bfloat16)                    # lhsT_z (last seg is a hi half)
    return consts


def stage_inputs(emissions, start, end, trans):
    """Host-side restaging: exp'ed bf16 emissions in per-core consumption
    layout + consts.  Returns (in_maps, c0, w0sum_unused)."""
    c0 = _calibrate_c0(emissions, start, trans)
    Ep = np.exp(trans.astype(np.float64) - c0).astype(ml_dtypes.bfloat16)
    consts = make_consts(Ep, end)

    in_maps = []
    for core in range(N_CORES):
        sl = slice(core * BL, (core + 1) * BL)
        w = np.exp(emissions[sl].astype(np.float32)).astype(ml_dtypes.bfloat16)
        arr = w.reshape(BL, C, Q, T)            # [b, c, k, t]
        init0 = np.exp(start.astype(np.float32)[None, :]
                       + emissions[sl, 0].astype(np.float32)
                       ).astype(ml_dtypes.bfloat16)   # [b, t]

        stg = np.zeros((P2, G * NSLOT, FD), ml_dtypes.bfloat16)
        for g in range(G):
            base = g * NSLOT
            # phase 1: stack j holds segs (16g+2j) lo, (16g+2j+1) hi
            lo = arr[:, 16 * g:16 * g + 16:2]    # [b, 8, k, t]
            hi = arr[:, 16 * g + 1:16 * g + 16:2]
            # [t, k, j, b] -> [t, k, j*b]
            stg[:T, base:base + Q] = lo.transpose(3, 2, 1, 0).reshape(T, Q, FD)
            stg[T:, base:base + Q] = hi.transpose(3, 2, 1, 0).reshape(T, Q, FD)
            # phase 2: head-stack q = 8g+j evolves segs (2q+1) lo, (2q+2) hi
            for j in range(SPG):
                q = SPG * g + j
                cs = slice(base + Q, base + NSLOT)
                bs = slice(j * BL, (j + 1) * BL)
                stg[:T, cs, bs] = arr[:, 2 * q + 1, :MH].transpose(2, 1, 0)
                if 2 * q + 2 < C:
                    stg[T:, cs, bs] = arr[:, 2 * q + 2, :MH].transpose(2, 1, 0)
                else:
                    stg[T:, cs, bs] = 1.0        # padding segment, ignored
        # seg 0 init (g=0, j=0, lo, slot 0) uses start_transitions
        stg[:T, 0, 0:BL] = init0.T
        # boot block: consts | g0 slots 0-1 | g1 slots 0-1 (single ramp DMA)
        bootb = np.zeros((P2, NBOOT * FD), ml_dtypes.bfloat16)
        bootb[:, :consts.shape[1]] = consts
        for g in range(G):
            bootb[:, (1 + 2 * g) * FD:(3 + 2 * g) * FD] = \
                stg[:, g * NSLOT:g * NSLOT + 2].reshape(P2, 2 * FD)
        in_maps.append({"wstg": stg, "boot": bootb})
    return in_maps, c0


def unpack_logZ(res_core, c0):
    """Recover logZ[BL] from one core's outputs (float64 host math)."""
    s_m1 = np.asarray(res_core["s_m1"], np.float64)    # [2, G*FD]
    s_end = np.asarray(res_core["s_end"], np.float64)
    s_2 = np.asarray(res_core["s_2"], np.float64)
    z = np.asarray(res_core["zraw"], np.float64)[0]    # [FD]

    def seg_col(c):
        st = c // 2
        return (c % 2), (st // SPG) * FD + (st % SPG) * BL

    def head_col(c):
        qq = (c - 1) // 2
        return 1 - (c % 2), (qq // SPG) * FD + (qq % SPG) * BL

    logZ = np.zeros(BL, np.float64)
    for c in range(C):
        r, col = seg_col(c)
        logZ += np.log(s_end[r, col:col + BL])
    for c in range(1, C):
        r2, col2 = head_col(c)
        r1, col1 = seg_col(c)
        rp, colp = seg_col(c - 1)
        logZ += (np.log(s_2[r2, col2:col2 + BL])
                 - np.log(s_end[rp, colp:colp + BL])
                 - np.log(s_m1[r1, col1:col1 + BL]))
    rl, coll = seg_col(C - 1)
    logZ += np.log(z[(SPG - 1) * BL:SPG * BL]) - np.log(s_end[rl, coll:coll + BL])
    return logZ + (S - 1) * c0


def _device_logZ(emissions, start, end, trans):
    global LAST_RESULTS
    nc = _get_program()
    in_maps, c0 = stage_inputs(emissions, start, end, trans)
    res = run_bass_kernel_spmd(
        nc, in_maps, core_ids=list(range(N_CORES)), trace=TRACE,
    )
    LAST_RESULTS = res
    logZ = np.empty(B, np.float32)
    for core in range(N_CORES):
        logZ[core * BL:(core + 1) * BL] = unpack_logZ(
            res.results[core], c0).astype(np.float32)
    return logZ


def _numpy_fallback(emissions, mask, start, end, trans):
    """Faithful float64 reference implementation (handles any mask)."""
    def fwd(use_mask):
        a = start[None, :].astype(np.float64) + emissions[:, 0].astype(np.float64)
        tr = trans.astype(np.float64)
        for t in range(1, emissions.shape[1]):
            inner = a[:, :, None] + tr[None] + emissions[:, t].astype(np.float64)[:, None, :]
            m = inner.max(axis=1, keepdims=True)
            new = np.log(np.exp(inner - m).sum(axis=1)) + m[:, 0, :]
            if use_mask:
                a = np.where(mask[:, t][:, None], new, a)
            else:
                a = new
        fin = a + end[None].astype(np.float64)
        m = fin.max(axis=1, keepdims=True)
        return np.log(np.exp(fin - m).sum(axis=1)) + m[:, 0]

    score = fwd(True)
    partition = fwd(False)
    return (partition - score).astype(np.float32)


def kernel(emissions, mask, start_transitions, end_transitions, transitions):
    emissions = np.asarray(emissions, dtype=np.float32)
    mask = np.asarray(mask)
    start = np.asarray(start_transitions, dtype=np.float32)
    end = np.asarray(end_transitions, dtype=np.float32)
    trans = np.asarray(transitions, dtype=np.float32)

    if not mask.all():
        return _numpy_fallback(emissions, mask, start, end, trans)

    # With an all-ones mask the masked recursion's where(mask, new, old) is
    # the identity, so score == partition; both come from the same forward
    # pass, computed on the 8 NeuronCores.
    logZ = _device_logZ(emissions, start, end, trans)
    partition = logZ
    score = logZ
    return (partition - score).astype(np.float32)
